# Initial kernel scaffold
#
"""HQQ grouped GEMM (MoE routing) on 8 TRN2 NeuronCores — v7.

v4 + SBUF-native host layouts: W, x are repacked on the host into the
exact SBUF tile layout ([128 partitions, contiguous free dim]) so every
DMA moves 128 long contiguous rows (~22 KB) instead of 1024 short ones
— on HW, DMA cost is descriptor-dominated (fixed cost per descriptor),
so this is ~8x fewer descriptors at full line rate.  y returns in the
staging layout and is unpacked on the host.  Weights stream in 4
kt-chunks and matmul emission is phased: the first 8 PSUM groups follow
the arriving chunks, the rest run back-to-back off resident weights.

Sharding (unchanged from v4): expert pairs (437,71)(417,83)(398,78) on
2 cores each (out halves), solos 306/258 on 1 core (full width); both
experts of a pair accumulate into one PSUM bank at disjoint column
ranges.  Host does dequant to bf16; device is pure DMA+matmul+evict.
"""
import sys
sys.path.insert(0, '/opt/trn_rl_repo')
import numpy as np

E, IN, OUT, GS, N = 8, 1024, 2816, 64, 2048
G = IN // GS
NC_ = 8
KT = IN // 128
P = 128
PSUM_FP32 = 512  # fp32 columns per PSUM bank
WB = 512         # out-cols per weight block (4 oc chunks, contiguous)


def _make_plan(counts):
    """Greedy: pair largest with smallest while total <=512 (pair -> 2
    cores, out halves); leftovers solo (1 core, full out).  If there are
    >=2 solos, out-chunks are donated from the heaviest solo to the
    lightest to balance their matmul streams.  Each core runs a list of
    independent SECTIONS {'experts', 'ow', 'o0'}."""
    counts = [int(c) for c in counts]
    avail = sorted(range(E), key=lambda e: -counts[e])
    groups = []
    while avail:
        big = avail[0]
        if len(avail) > 1 and counts[big] + counts[avail[-1]] <= PSUM_FP32:
            groups.append({'experts': [big, avail[-1]], 'ow': OUT // 2})
        else:
            groups.append({'experts': [big], 'ow': OUT})
        avail = [e for e in avail if e not in groups[-1]['experts']]
    plan = []  # per core: list of sections
    solos = []
    for g in groups:
        for s in range(OUT // g['ow']):
            plan.append([{'experts': g['experts'], 'ow': g['ow'],
                          'o0': s * g['ow']}])
            if len(g['experts']) == 1 and g['ow'] == OUT:
                solos.append(len(plan) - 1)
    assert len(plan) == NC_, f"plan used {len(plan)} cores"
    if len(solos) >= 2:
        solos.sort(key=lambda i: -counts[plan[i][0]['experts'][0]])
        heavy, light = solos[0], solos[-1]
        ch = counts[plan[heavy][0]['experts'][0]]
        cl = counts[plan[light][0]['experts'][0]]
        noc = OUT // P
        # balance ch*(noc-d) ~= cl*noc + ch*d  ->  d = noc*(ch-cl)/(2*ch)
        # recipient is bytes-bound: each donated oc costs it ~2.5us of
        # serialized DMA+PE, so floor the pure-FLOP balance
        d = int(noc * (ch - cl) / (2 * ch))
        d = max(0, min(d, noc - 1))
        if d > 0:
            eh = plan[heavy][0]['experts'][0]
            plan[heavy][0]['ow'] = OUT - d * P
            plan[light].append({'experts': [eh], 'ow': d * P,
                                'o0': OUT - d * P})
    return plan


def _token_chunks(cnts):
    """Split the concatenated token axis into <=512 chunks of
    (expert, expert_local_t0, chunk_local_t0, len) runs."""
    chunks, cur, cur_len, ct0 = [], [], 0, 0
    for e in range(len(cnts)):
        left, lt0 = int(cnts[e]), 0
        while left:
            take = min(left, PSUM_FP32 - cur_len)
            cur.append((e, lt0, cur_len, take))
            cur_len += take
            lt0 += take
            left -= take
            if cur_len == PSUM_FP32:
                chunks.append((ct0, cur_len, cur))
                ct0 += cur_len
                cur, cur_len = [], 0
    if cur_len:
        chunks.append((ct0, cur_len, cur))
    return chunks


def _build_prog(key, reps=1):
    """One Bass program of independent sections; key = ((cnts, ow), ...).
    Per section s (SBUF-native layouts):
      xp{s} [128, KT*T]    xp[p, kt*T + t]           = x[tok t, kt*128+p]
      Wp{s} [128, ...]     block layout, see _wblocks
      yp{s} [128, noc*T]   yp[p, oc*T + t]           = y[tok t, oc*128+p]
    """
    import concourse.mybir as mybir
    from concourse import bacc
    from concourse.tile import TileContext

    nc = bacc.Bacc('TRN2')
    dt = mybir.dt
    secs = []
    for si, (cnts, ow) in enumerate(key):
        nexp = len(cnts)
        T = int(sum(cnts))
        noc = ow // P
        secs.append({
            'cnts': cnts, 'ow': ow, 'nexp': nexp, 'T': T, 'noc': noc,
            'chunks': _token_chunks(cnts), 'wbs': _wblocks(ow),
            'xp': nc.dram_tensor(f"xp{si}", [P, KT * T], dt.bfloat16,
                                 kind="ExternalInput"),
            'Wp': nc.dram_tensor(f"Wp{si}", [P, KT * nexp * ow],
                                 dt.bfloat16, kind="ExternalInput"),
            'yp': nc.dram_tensor(f"yp{si}", [P, noc * T], dt.bfloat16,
                                 kind="ExternalOutput"),
        })

    with TileContext(nc) as tc:
        with tc.tile_pool(name="persist", bufs=1) as pp, \
             tc.tile_pool(name="psum", bufs=8, space="PSUM") as ps:

            for si, s in enumerate(secs):
                s['xall'] = pp.tile([P, KT * s['T']], dt.bfloat16,
                                    name=f"xall{si}", tag=f"xall{si}")
                s['wall'] = pp.tile([P, KT * s['nexp'] * s['ow']],
                                    dt.bfloat16, name=f"wall{si}",
                                    tag=f"wall{si}")
                s['ysb'] = pp.tile([P, s['noc'] * s['T']], dt.bfloat16,
                                   name=f"ysb{si}", tag=f"ysb{si}")

            for _rep in range(reps):
                # inputs: section 0's first W block, then all later
                # sections' (small) inputs, then section 0's remaining
                # blocks — so late sections' weights are resident long
                # before their matmuls run at the end.
                s0 = secs[0]
                _emit_inputs(nc, s0['xp'], s0['Wp'], s0['xall'],
                             s0['wall'], s0['wbs'], s0['nexp'], s0['T'],
                             (0, 1))
                for s in secs[1:]:
                    _emit_inputs(nc, s['xp'], s['Wp'], s['xall'],
                                 s['wall'], s['wbs'], s['nexp'], s['T'],
                                 (0, len(s['wbs'])))
                _emit_inputs(nc, s0['xp'], s0['Wp'], s0['xall'],
                             s0['wall'], s0['wbs'], s0['nexp'], s0['T'],
                             (1, len(s0['wbs'])))
                for s in secs:
                    _emit_compute(nc, dt, ps, s['yp'], s['xall'],
                                  s['wall'], s['ysb'], s['chunks'],
                                  s['wbs'], s['nexp'], s['ow'],
                                  s['noc'], s['T'])
    nc.compile()
    return nc


def _wblocks(ow):
    """Weight layout blocks: [(o0, bw, coloff)] — block holds cols
    [o0, o0+bw) of every (kt, expert), packed contiguously at coloff.
    Graded sizes: small first blocks so the PE starts early."""
    sizes = []
    rem = ow
    if rem > 128:
        sizes.append(128)
        rem -= 128
    while rem > 256:
        sizes.append(256)
        rem -= 256
    if rem:
        sizes.append(rem)
    wbs = []
    o0 = 0
    coloff = 0
    for bw in sizes:
        wbs.append((o0, bw, coloff))
        o0 += bw
        coloff += KT * bw  # per expert accounted in nexp factor at use site
    return wbs


def _emit_inputs(nc, xp, Wp, xall, wall_, wbs, nexp, T, wrange):
    # x: three contiguous line-rate DMAs on the ACT ring (kt0, kt1-3,
    # kt4-7) so the first matmuls are gated on minimal x transfer
    if wrange[0] == 0:
        for a, b in ((0, T), (T, 4 * T), (4 * T, KT * T)):
            nc.scalar.dma_start(xall[:, a:b], xp.ap()[:, a:b])
    # W: one contiguous DMA per oc-block on the SP ring (128 descriptors
    # each); PSUM groups of block b complete as soon as block b lands.
    for (o0, bw, coloff) in wbs[wrange[0]:wrange[1]]:
        a, b = coloff * nexp, (coloff + KT * bw) * nexp
        nc.sync.dma_start(wall_[:, a:b], Wp.ap()[:, a:b])


def _emit_compute(nc, dt, ps, yp, xall, wall_, ysb,
                  chunks, wbs, nexp, ow, noc, T):
    def wslice(e, kt, oc):
        # col offset of [oc*128, oc*128+128) for (e, kt) in block layout
        for (o0, bw, coloff) in wbs:
            if o0 <= oc * P < o0 + bw:
                base = coloff * nexp + (kt * nexp + e) * bw + (oc * P - o0)
                return wall_[:, base:base + P]
        raise AssertionError

    # simple oc-major emission: groups complete progressively behind the
    # weight stream; y DMAs stream on the SP ring behind the weights.
    total = noc * T
    ypos = 0
    yprev = 0
    for oc in range(noc):
        occhunks = chunks
        if len(chunks) == 1 and oc == noc - 1 and chunks[0][1] > 128:
            # split the final group into token halves: the first half's
            # evict+flush overlaps the second half's matmuls, shortening
            # the end-of-kernel serial tail
            fc0, fclen, fcruns = chunks[0]
            h = fclen // 2

            def _clip(a, b):
                out = []
                for (e, lt0, co, ln) in fcruns:
                    s, t = max(co, a), min(co + ln, b)
                    if s < t:
                        out.append((e, lt0 + (s - co), s - a, t - s))
                return out
            occhunks = [(fc0, h, _clip(0, h)),
                        (fc0 + h, fclen - h, _clip(h, fclen))]
        for ci, (c0, clen, cruns) in enumerate(occhunks):
            pt = ps.tile([P, clen], dt.float32, name="pt", tag="pt")
            for kt in range(KT):
                for ri, (e, lt0, co, ln) in enumerate(cruns):
                    nc.tensor.matmul(
                        pt[:, co:co + ln], wslice(e, kt, oc),
                        xall[:, kt * T + c0 + co:kt * T + c0 + co + ln],
                        start=(kt == 0 and ri == 0),
                        stop=(kt == KT - 1 and ri == 0),
                        skip_group_check=(ri > 0))
            nc.vector.tensor_copy(
                ysb[:, oc * T + c0:oc * T + c0 + clen], pt[:, :])
            ypos += clen
        # flush y after 6 ocs, then every ~3, with fine final flushes so
        # the post-matmul tail is minimal
        left = noc - 1 - oc
        if (ypos == total or (yprev == 0 and ypos >= 6 * T)
                or (yprev > 0 and left > 2 and ypos - yprev >= 3 * T)
                or (0 < left <= 2 and ypos - yprev >= T)):
            nc.sync.dma_start(yp.ap()[:, yprev:ypos], ysb[:, yprev:ypos])
            yprev = ypos
    if yprev < total:
        nc.sync.dma_start(yp.ap()[:, yprev:total], ysb[:, yprev:total])


# ---------------------------------------------------------------- host side

def _host_prep(input, tokens_per_expert, W_q, scales, zeros):
    import ml_dtypes
    x = np.asarray(input, dtype=np.float32)
    counts = np.asarray(tokens_per_expert, dtype=np.int32)
    Wq = np.asarray(W_q, dtype=np.int32)
    sc = np.asarray(scales, dtype=np.float32)
    zr = np.asarray(zeros, dtype=np.float32)

    Wf = (Wq.reshape(E, G, GS, OUT).astype(np.float32) - zr[:, :, None, :]) \
        * sc[:, :, None, :]
    W = Wf.reshape(E, IN, OUT).astype(ml_dtypes.bfloat16)
    xTb = np.ascontiguousarray(x.T).astype(ml_dtypes.bfloat16)
    return xTb, W, counts


def _core_specs(plan, counts, xTb, W):
    starts = np.concatenate([[0], np.cumsum(counts)]).astype(int)
    specs = []
    for sections in plan:
        key = []
        in_map = {}
        for si, p in enumerate(sections):
            exps = p['experts']
            cnts = tuple(int(counts[e]) for e in exps)
            key.append((cnts, p['ow']))
            xcols = np.concatenate(
                [np.arange(starts[e], starts[e] + counts[e]) for e in exps])
            T = len(xcols)
            xsel = xTb[:, xcols]                       # [IN, T]
            xp = np.ascontiguousarray(
                xsel.reshape(KT, P, T).transpose(1, 0, 2).reshape(P, KT * T))
            ow, o0 = p['ow'], p['o0']
            ws = np.stack([W[e][:, o0:o0 + ow] for e in exps])
            wkpo = ws.reshape(len(exps), KT, P, ow)
            blocks = []
            for (b0, bw, coloff) in _wblocks(ow):
                blk = wkpo[:, :, :, b0:b0 + bw]        # [e, kt, p, bw]
                blocks.append(blk.transpose(2, 1, 0, 3).reshape(P, -1))
            in_map[f"xp{si}"] = xp
            in_map[f"Wp{si}"] = np.ascontiguousarray(
                np.concatenate(blocks, axis=1))
        specs.append({'key': tuple(key), 'in_map': in_map,
                      'plan': sections})
    return specs


def _assemble(plan, counts, results):
    starts = np.concatenate([[0], np.cumsum(counts)]).astype(int)
    y = np.empty((N, OUT), np.float32)
    for sections, res in zip(plan, results):
        for si, p in enumerate(sections):
            ow = p['ow']
            noc = ow // P
            exps = p['experts']
            T = int(sum(counts[e] for e in exps))
            yp = np.asarray(res[f"yp{si}"]).astype(np.float32)
            yT = yp.reshape(P, noc, T).transpose(1, 0, 2).reshape(ow, T)
            lt0 = 0
            for e in exps:
                ce = int(counts[e])
                y[starts[e]:starts[e] + ce, p['o0']:p['o0'] + ow] = \
                    yT[:, lt0:lt0 + ce].T
                lt0 += ce
    return y


# ---------------------------------------------------------------- runner

def _io_spec(nc):
    import concourse.mybir as mybir
    import jax
    in_names, out_names, out_avals, zero_outs = [], [], [], []
    for alloc in nc.m.functions[0].allocations:
        if not isinstance(alloc, mybir.MemoryLocationSet):
            continue
        name = alloc.memorylocations[0].name
        if alloc.kind == "ExternalInput":
            in_names.append(name)
        elif alloc.kind == "ExternalOutput":
            out_names.append(name)
            shape = tuple(alloc.tensor_shape)
            dtype = mybir.dt.np(alloc.dtype)
            out_avals.append(jax.core.ShapedArray(shape, dtype))
            zero_outs.append(np.zeros(shape, dtype))
    return in_names, out_names, out_avals, zero_outs


def _make_jit(nc):
    import jax
    from concourse import bass2jax
    in_names, out_names, out_avals, zero_outs = _io_spec(nc)
    assert nc.dbg_addr is None
    partition_name = (nc.partition_id_tensor.name
                      if nc.partition_id_tensor else None)
    in_names = [n for n in in_names if n != partition_name]
    nparams = len(in_names)
    all_names = in_names + out_names
    if partition_name is not None:
        all_names = all_names + [partition_name]

    def _body(*args):
        operands = list(args)
        if partition_name is not None:
            operands.append(bass2jax.partition_id_tensor())
        outs = bass2jax._bass_exec_p.bind(
            *operands,
            out_avals=tuple(out_avals),
            in_names=tuple(all_names),
            out_names=tuple(out_names),
            lowering_input_output_aliases=(),
            sim_require_finite=True,
            sim_require_nnan=True,
            nc=nc,
        )
        return tuple(outs)

    donate = tuple(range(nparams, nparams + len(out_avals)))
    return (jax.jit(_body, donate_argnums=donate, keep_unused=True),
            in_names, out_names, zero_outs)


def _run_multi(specs, ncs):
    import jax
    from concourse import bass2jax
    bass2jax.install_neuronx_cc_hook()
    devices = jax.devices()
    fns = {key: _make_jit(nc) for key, nc in ncs.items()}
    pending = []
    for i, spec in enumerate(specs):
        fn, in_names, out_names, zero_outs = fns[spec['key']]
        dev = devices[i]
        args = [jax.device_put(np.asarray(spec['in_map'][n]), dev)
                for n in in_names]
        args += [jax.device_put(z, dev) for z in zero_outs]
        pending.append((fn(*args), out_names))
    return [{nm: np.asarray(o) for nm, o in zip(out_names, outs)}
            for outs, out_names in pending]


# ---------------------------------------------------------------- entry

def kernel(input, tokens_per_expert, W_q, scales, zeros):
    xTb, W, counts = _host_prep(input, tokens_per_expert, W_q, scales, zeros)
    plan = _make_plan(counts)
    specs = _core_specs(plan, counts, xTb, W)
    ncs = {}
    for spec in specs:
        key = spec['key']
        if key not in ncs:
            ncs[key] = _build_prog(key)
    results = _run_multi(specs, ncs)
    return _assemble(plan, counts, results)



# revision 3
# speedup vs baseline: 1.2445x; 1.2445x over previous
"""HQQ grouped GEMM (MoE routing) on 8 TRN2 NeuronCores — v8 (fp8 DoubleRow).

v7 structure, but the matmul runs in fp8e4m3 DoubleRow perf mode (0.5
cycles/row, 256-deep contraction per instruction).  Precision is kept at
(better-than-)bf16 parity with a 3-term split:

    y = x_hi @ W_hi + x_lo @ W_hi + x_hi @ W_lo

where x = x_hi + x_lo and W·alpha = W_hi + W_lo are exact two-word fp8
decompositions.  alpha is a per-output-column power of two chosen on the
host so every W column sits in e4m3's normal range (HQQ group scales span
20x, which otherwise lands small-scale groups in fp8 subnormals); it is
undone at PSUM-evict time by a per-partition tensor_scalar_mul (PSUM
partition axis == output column).  Each DoubleRow instruction packs one
term for a kt-PAIR (A=W[2j], B=W[2j+1]), so per kt-pair a PSUM group takes
3 instructions x 0.5 cycles/row = 0.75x the bf16 row count.

DMA: W_hi streams on the sync (SP) queue, W_lo on the scalar (ACT) queue,
x/inva/y on the gpsimd (Pool) queue — the cost model gives each queue its
own full-rate DMA engine set, so ~8.3MB of traffic hides behind ~14us of
PE time on the heaviest core.

Sharding (unchanged from v4): expert pairs on 2 cores each (out halves),
solos on 1 core; both experts of a pair accumulate into one PSUM bank at
disjoint column ranges.  Host does dequant + fp8 split; device is pure
DMA+matmul+scaled-evict.
"""
import sys
sys.path.insert(0, '/opt/trn_rl_repo')
import numpy as np

E, IN, OUT, GS, N = 8, 1024, 2816, 64, 2048
G = IN // GS
NC_ = 8
KT = IN // 128
NJ = KT // 2     # kt pairs per contraction
P = 128
PSUM_FP32 = 512  # fp32 columns per PSUM bank


def _make_plan(counts):
    """Greedy: pair largest with smallest while total <=512 (pair -> 2
    cores, out halves); leftovers solo (1 core, full out).  If there are
    >=2 solos, out-chunks are donated from the heaviest solo to the
    lightest to balance their matmul streams.  Each core runs a list of
    independent SECTIONS {'experts', 'ow', 'o0'}."""
    counts = [int(c) for c in counts]
    avail = sorted(range(E), key=lambda e: -counts[e])
    groups = []
    while avail:
        big = avail[0]
        if len(avail) > 1 and counts[big] + counts[avail[-1]] <= PSUM_FP32:
            groups.append({'experts': [big, avail[-1]], 'ow': OUT // 2})
        else:
            groups.append({'experts': [big], 'ow': OUT})
        avail = [e for e in avail if e not in groups[-1]['experts']]
    plan = []  # per core: list of sections
    solos = []
    for g in groups:
        for s in range(OUT // g['ow']):
            plan.append([{'experts': g['experts'], 'ow': g['ow'],
                          'o0': s * g['ow']}])
            if len(g['experts']) == 1 and g['ow'] == OUT:
                solos.append(len(plan) - 1)
    assert len(plan) == NC_, f"plan used {len(plan)} cores"
    if len(solos) >= 2:
        solos.sort(key=lambda i: -counts[plan[i][0]['experts'][0]])
        heavy, light = solos[0], solos[-1]
        ch = counts[plan[heavy][0]['experts'][0]]
        cl = counts[plan[light][0]['experts'][0]]
        noc = OUT // P
        # balance ch*(noc-d) ~= cl*noc + ch*d  ->  d = noc*(ch-cl)/(2*ch)
        d = int(noc * (ch - cl) / (2 * ch))
        d = max(0, min(d, noc - 1))
        if d > 0:
            eh = plan[heavy][0]['experts'][0]
            plan[heavy][0]['ow'] = OUT - d * P
            plan[light].append({'experts': [eh], 'ow': d * P,
                                'o0': OUT - d * P})
    return plan


def _token_chunks(cnts):
    """Split the concatenated token axis into <=512 chunks of
    (expert, expert_local_t0, chunk_local_t0, len) runs."""
    chunks, cur, cur_len, ct0 = [], [], 0, 0
    for e in range(len(cnts)):
        left, lt0 = int(cnts[e]), 0
        while left:
            take = min(left, PSUM_FP32 - cur_len)
            cur.append((e, lt0, cur_len, take))
            cur_len += take
            lt0 += take
            left -= take
            if cur_len == PSUM_FP32:
                chunks.append((ct0, cur_len, cur))
                ct0 += cur_len
                cur, cur_len = [], 0
    if cur_len:
        chunks.append((ct0, cur_len, cur))
    return chunks


def _wblocks(ow):
    """Weight layout blocks: [(o0, bw, coloff)] — block holds cols
    [o0, o0+bw) of every (kt, expert), packed contiguously at coloff.
    Graded sizes: small first blocks so the PE starts early."""
    sizes = []
    rem = ow
    if rem > 128:
        sizes.append(128)
        rem -= 128
    while rem > 256:
        sizes.append(256)
        rem -= 256
    if rem:
        sizes.append(rem)
    wbs = []
    o0 = 0
    coloff = 0
    for bw in sizes:
        wbs.append((o0, bw, coloff))
        o0 += bw
        coloff += KT * bw  # per expert accounted in nexp factor at use site
    return wbs


def _build_prog(key, reps=1):
    """One Bass program of independent sections; key = ((cnts, ow), ...).
    Per section s (SBUF-native layouts, fp8e4m3):
      xp{s}  [128, 2*KT*T]  xp[p, h*KT*T + kt*T + t] = xh/xl[tok t, kt*128+p]
      Whi{s} [128, KT*nexp*ow]  block layout, see _wblocks
      Wlo{s} [128, KT*nexp*ow]  same layout (fp8 residual of alpha-scaled W)
      ia{s}  [128, noc] fp32    ia[p, oc] = 1/alpha[o0 + oc*128 + p]
      yp{s}  [128, noc*T] bf16  yp[p, oc*T + t] = y[tok t, oc*128+p]
    """
    import concourse.mybir as mybir
    from concourse import bacc
    from concourse.tile import TileContext

    nc = bacc.Bacc('TRN2')
    dt = mybir.dt
    secs = []
    for si, (cnts, ow) in enumerate(key):
        nexp = len(cnts)
        T = int(sum(cnts))
        noc = ow // P
        secs.append({
            'cnts': cnts, 'ow': ow, 'nexp': nexp, 'T': T, 'noc': noc,
            'chunks': _token_chunks(cnts), 'wbs': _wblocks(ow),
            'xp': nc.dram_tensor(f"xp{si}", [P, 2 * KT * T], dt.float8e4,
                                 kind="ExternalInput"),
            'Whi': nc.dram_tensor(f"Whi{si}", [P, KT * nexp * ow],
                                  dt.float8e4, kind="ExternalInput"),
            'Wlo': nc.dram_tensor(f"Wlo{si}", [P, KT * nexp * ow],
                                  dt.float8e4, kind="ExternalInput"),
            'ia': nc.dram_tensor(f"ia{si}", [P, noc], dt.float32,
                                 kind="ExternalInput"),
            'yp': nc.dram_tensor(f"yp{si}", [P, noc * T], dt.bfloat16,
                                 kind="ExternalOutput"),
        })

    with TileContext(nc) as tc:
        with tc.tile_pool(name="persist", bufs=1) as pp, \
             tc.tile_pool(name="psum", bufs=8, space="PSUM") as ps:

            for si, s in enumerate(secs):
                s['xall'] = pp.tile([P, 2 * KT * s['T']], dt.float8e4,
                                    name=f"xall{si}", tag=f"xall{si}")
                s['whi'] = pp.tile([P, KT * s['nexp'] * s['ow']],
                                   dt.float8e4, name=f"whi{si}",
                                   tag=f"whi{si}")
                s['wlo'] = pp.tile([P, KT * s['nexp'] * s['ow']],
                                   dt.float8e4, name=f"wlo{si}",
                                   tag=f"wlo{si}")
                s['iat'] = pp.tile([P, s['noc']], dt.float32,
                                   name=f"iat{si}", tag=f"iat{si}")
                s['ysb'] = pp.tile([P, s['noc'] * s['T']], dt.bfloat16,
                                   name=f"ysb{si}", tag=f"ysb{si}")

            for _rep in range(reps):
                # inputs: section 0's first W block, then all later
                # sections' (small) inputs, then section 0's remaining
                # blocks — so late sections' weights are resident long
                # before their matmuls run at the end.
                s0 = secs[0]
                _emit_inputs(nc, s0, (0, 1))
                for s in secs[1:]:
                    _emit_inputs(nc, s, (0, len(s['wbs'])))
                _emit_inputs(nc, s0, (1, len(s0['wbs'])))
                for s in secs:
                    _emit_compute(nc, dt, ps, s)
    nc.compile()
    return nc


def _emit_inputs(nc, s, wrange):
    T, nexp = s['T'], s['nexp']
    if wrange[0] == 0:
        # x (hi then lo interleaved by kt-pair need): first kt pair of
        # both halves lands first so the j=0 matmuls are gated on minimal
        # transfer; inva rides along (needed at first evict).
        H = KT * T
        xp, xall = s['xp'], s['xall']
        nc.gpsimd.dma_start(s['iat'][:, :], s['ia'].ap()[:, :])
        for a, b in ((0, 2 * T), (H, H + 2 * T), (2 * T, H),
                     (H + 2 * T, 2 * H)):
            if b > a:
                nc.gpsimd.dma_start(xall[:, a:b], xp.ap()[:, a:b])
    # W: one contiguous DMA per oc-block; W_hi on the sync ring, W_lo on
    # the scalar ring.  PSUM groups of block b complete once b lands.
    for (o0, bw, coloff) in s['wbs'][wrange[0]:wrange[1]]:
        a, b = coloff * nexp, (coloff + KT * bw) * nexp
        nc.sync.dma_start(s['whi'][:, a:b], s['Whi'].ap()[:, a:b])
        nc.scalar.dma_start(s['wlo'][:, a:b], s['Wlo'].ap()[:, a:b])


def _emit_compute(nc, dt, ps, s):
    import concourse.mybir as mybir
    chunks, wbs = s['chunks'], s['wbs']
    nexp, ow, noc, T = s['nexp'], s['ow'], s['noc'], s['T']
    xall, whi, wlo, iat, ysb = (s['xall'], s['whi'], s['wlo'], s['iat'],
                                s['ysb'])
    H = KT * T

    def wpair(wall, e, j, oc):
        # DoubleRow weights AP [128, 2, 128]: [:, i, :] = cols
        # [oc*128, oc*128+128) of (kt=2j+i, e) in the block layout
        for (o0, bw, coloff) in wbs:
            if o0 <= oc * P < o0 + bw:
                a = coloff * nexp + (2 * j) * nexp * bw
                seg = wall[:, a:a + 2 * nexp * bw]
                s3 = seg.rearrange("p (two c) -> p two c", two=2)
                off = e * bw + (oc * P - o0)
                return s3[:, :, off:off + P]
        raise AssertionError

    def xpair(h, j, c0, clen):
        # DoubleRow ifmap AP [128, 2, clen]: [:, i, :] = xh/xl tokens
        # [c0, c0+clen) of kt=2j+i
        a = h * H + 2 * j * T
        seg = xall[:, a:a + 2 * T]
        s3 = seg.rearrange("p (two c) -> p two c", two=2)
        return s3[:, :, c0:c0 + clen]

    # simple oc-major emission: groups complete progressively behind the
    # weight stream; y DMAs stream on the gpsimd ring behind x.
    total = noc * T
    ypos = 0
    yprev = 0
    for oc in range(noc):
        occhunks = chunks
        if len(chunks) == 1 and oc == noc - 1 and chunks[0][1] > 128:
            # split the final group into token halves: the first half's
            # evict+flush overlaps the second half's matmuls, shortening
            # the end-of-kernel serial tail
            fc0, fclen, fcruns = chunks[0]
            h = fclen // 2

            def _clip(a, b):
                out = []
                for (e, lt0, co, ln) in fcruns:
                    sa, tb = max(co, a), min(co + ln, b)
                    if sa < tb:
                        out.append((e, lt0 + (sa - co), sa - a, tb - sa))
                return out
            occhunks = [(fc0, h, _clip(0, h)),
                        (fc0 + h, fclen - h, _clip(h, fclen))]
        for ci, (c0, clen, cruns) in enumerate(occhunks):
            pt = ps.tile([P, clen], dt.float32, name="pt", tag="pt")
            for j in range(NJ):
                for term in range(3):
                    wall = whi if term < 2 else wlo
                    h = 1 if term == 1 else 0
                    for ri, (e, lt0, co, ln) in enumerate(cruns):
                        nc.tensor.matmul(
                            pt[:, co:co + ln], wpair(wall, e, j, oc),
                            xpair(h, j, c0 + co, ln),
                            start=(j == 0 and term == 0 and ri == 0),
                            stop=(j == NJ - 1 and term == 2 and ri == 0),
                            perf_mode=mybir.MatmulPerfMode.DoubleRow,
                            skip_group_check=(ri > 0))
            nc.vector.tensor_scalar_mul(
                ysb[:, oc * T + c0:oc * T + c0 + clen], pt[:, :],
                iat[:, oc:oc + 1])
            ypos += clen
        # flush y after 6 ocs, then every ~3, with fine final flushes so
        # the post-matmul tail is minimal
        left = noc - 1 - oc
        if (ypos == total or (yprev == 0 and ypos >= 6 * T)
                or (yprev > 0 and left > 2 and ypos - yprev >= 3 * T)
                or (0 < left <= 2 and ypos - yprev >= T)):
            nc.gpsimd.dma_start(s['yp'].ap()[:, yprev:ypos],
                                ysb[:, yprev:ypos])
            yprev = ypos
    if yprev < total:
        nc.gpsimd.dma_start(s['yp'].ap()[:, yprev:total], ysb[:, yprev:total])


# ---------------------------------------------------------------- host side

def _host_prep(input, tokens_per_expert, W_q, scales, zeros):
    x = np.asarray(input, dtype=np.float32)
    counts = np.asarray(tokens_per_expert, dtype=np.int32)
    Wq = np.asarray(W_q, dtype=np.int32)
    sc = np.asarray(scales, dtype=np.float32)
    zr = np.asarray(zeros, dtype=np.float32)

    Wf = (Wq.reshape(E, G, GS, OUT).astype(np.float32) - zr[:, :, None, :]) \
        * sc[:, :, None, :]
    W = Wf.reshape(E, IN, OUT)          # fp32; fp8 split happens per section
    xT = np.ascontiguousarray(x.T)      # [IN, N] fp32
    return xT, W, counts


def _core_specs(plan, counts, xT, W):
    import ml_dtypes
    f8 = ml_dtypes.float8_e4m3
    starts = np.concatenate([[0], np.cumsum(counts)]).astype(int)
    specs = []
    for sections in plan:
        key = []
        in_map = {}
        for si, p in enumerate(sections):
            exps = p['experts']
            cnts = tuple(int(counts[e]) for e in exps)
            key.append((cnts, p['ow']))
            xcols = np.concatenate(
                [np.arange(starts[e], starts[e] + counts[e]) for e in exps])
            T = len(xcols)
            xsel = xT[:, xcols]                        # [IN, T] fp32
            xh = xsel.astype(f8)
            xl = (xsel - xh.astype(np.float32)).astype(f8)
            xs = np.stack([xh, xl])                    # [2, IN, T]
            xp = np.ascontiguousarray(
                xs.reshape(2, KT, P, T).transpose(2, 0, 1, 3)
                .reshape(P, 2 * KT * T))
            ow, o0 = p['ow'], p['o0']
            ws = np.stack([W[e][:, o0:o0 + ow] for e in exps])  # [e,IN,ow]
            colmax = np.abs(ws).max(axis=(0, 1))       # [ow]
            colmax = np.maximum(colmax, 1e-30)
            alpha = 2.0 ** np.floor(np.log2(224.0 / colmax))
            wn = ws * alpha[None, None, :]
            wh = wn.astype(f8)
            wl = (wn - wh.astype(np.float32)).astype(f8)
            blocks_h, blocks_l = [], []
            for arr, blocks in ((wh, blocks_h), (wl, blocks_l)):
                wkpo = arr.reshape(len(exps), KT, P, ow)
                for (b0, bw, coloff) in _wblocks(ow):
                    blk = wkpo[:, :, :, b0:b0 + bw]    # [e, kt, p, bw]
                    blocks.append(blk.transpose(2, 1, 0, 3).reshape(P, -1))
            ia = (1.0 / alpha).reshape(ow // P, P).T   # [P, noc]
            in_map[f"xp{si}"] = xp
            in_map[f"Whi{si}"] = np.ascontiguousarray(
                np.concatenate(blocks_h, axis=1))
            in_map[f"Wlo{si}"] = np.ascontiguousarray(
                np.concatenate(blocks_l, axis=1))
            in_map[f"ia{si}"] = np.ascontiguousarray(ia.astype(np.float32))
        specs.append({'key': tuple(key), 'in_map': in_map,
                      'plan': sections})
    return specs


def _assemble(plan, counts, results):
    starts = np.concatenate([[0], np.cumsum(counts)]).astype(int)
    y = np.empty((N, OUT), np.float32)
    for sections, res in zip(plan, results):
        for si, p in enumerate(sections):
            ow = p['ow']
            noc = ow // P
            exps = p['experts']
            T = int(sum(counts[e] for e in exps))
            yp = np.asarray(res[f"yp{si}"]).astype(np.float32)
            yT = yp.reshape(P, noc, T).transpose(1, 0, 2).reshape(ow, T)
            lt0 = 0
            for e in exps:
                ce = int(counts[e])
                y[starts[e]:starts[e] + ce, p['o0']:p['o0'] + ow] = \
                    yT[:, lt0:lt0 + ce].T
                lt0 += ce
    return y


# ---------------------------------------------------------------- runner

def _io_spec(nc):
    import concourse.mybir as mybir
    import jax
    in_names, out_names, out_avals, zero_outs = [], [], [], []
    for alloc in nc.m.functions[0].allocations:
        if not isinstance(alloc, mybir.MemoryLocationSet):
            continue
        name = alloc.memorylocations[0].name
        if alloc.kind == "ExternalInput":
            in_names.append(name)
        elif alloc.kind == "ExternalOutput":
            out_names.append(name)
            shape = tuple(alloc.tensor_shape)
            dtype = mybir.dt.np(alloc.dtype)
            out_avals.append(jax.core.ShapedArray(shape, dtype))
            zero_outs.append(np.zeros(shape, dtype))
    return in_names, out_names, out_avals, zero_outs


def _make_jit(nc):
    import jax
    from concourse import bass2jax
    in_names, out_names, out_avals, zero_outs = _io_spec(nc)
    assert nc.dbg_addr is None
    partition_name = (nc.partition_id_tensor.name
                      if nc.partition_id_tensor else None)
    in_names = [n for n in in_names if n != partition_name]
    nparams = len(in_names)
    all_names = in_names + out_names
    if partition_name is not None:
        all_names = all_names + [partition_name]

    def _body(*args):
        operands = list(args)
        if partition_name is not None:
            operands.append(bass2jax.partition_id_tensor())
        outs = bass2jax._bass_exec_p.bind(
            *operands,
            out_avals=tuple(out_avals),
            in_names=tuple(all_names),
            out_names=tuple(out_names),
            lowering_input_output_aliases=(),
            sim_require_finite=True,
            sim_require_nnan=True,
            nc=nc,
        )
        return tuple(outs)

    donate = tuple(range(nparams, nparams + len(out_avals)))
    return (jax.jit(_body, donate_argnums=donate, keep_unused=True),
            in_names, out_names, zero_outs)


def _run_multi(specs, ncs):
    import jax
    from concourse import bass2jax
    bass2jax.install_neuronx_cc_hook()
    devices = jax.devices()
    fns = {key: _make_jit(nc) for key, nc in ncs.items()}
    pending = []
    for i, spec in enumerate(specs):
        fn, in_names, out_names, zero_outs = fns[spec['key']]
        dev = devices[i]
        args = [jax.device_put(np.asarray(spec['in_map'][n]), dev)
                for n in in_names]
        args += [jax.device_put(z, dev) for z in zero_outs]
        pending.append((fn(*args), out_names))
    return [{nm: np.asarray(o) for nm, o in zip(out_names, outs)}
            for outs, out_names in pending]


# ---------------------------------------------------------------- entry

def kernel(input, tokens_per_expert, W_q, scales, zeros):
    xT, W, counts = _host_prep(input, tokens_per_expert, W_q, scales, zeros)
    plan = _make_plan(counts)
    specs = _core_specs(plan, counts, xT, W)
    ncs = {}
    for spec in specs:
        key = spec['key']
        if key not in ncs:
            ncs[key] = _build_prog(key)
    results = _run_multi(specs, ncs)
    return _assemble(plan, counts, results)
